# revision 16
# baseline (speedup 1.0000x reference)
"""Pointer-generator output layer on 8 Trainium2 NeuronCores (Bass/Tile).

Algorithm (per reference.py):
  interp = sigmoid(x @ Wp + bp)                              (B, 1)
  gen    = softmax(x @ Wg + bg)  scattered via gen_to_out    (B, OUT_V)
  ptr    = softmax(scores)       scattered via inp_to_out[ctx_inp]
  out    = interp * gen + (1 - interp) * ptr

Sharding: output-vocab sharded across 8 cores. Host sorts the (static,
shared) index tables so both scatters become monotone "staircase" 0/1
matmuls on device:
  - Wg columns permuted by argsort(gen_to_out): each core reads only the
    Wg columns mapping into its vocab slice; softmax stats all-reduced.
  - inp_to_out sorted; ctx_inp relabeled by rank (host int gather). On
    device a per-row histogram over the 20000 rank bins is built with
    one-hot matmuls accumulating in PSUM (exact fp32), batch-sharded;
    the per-rank histograms are exchanged (AllToAll) and applied through
    the sorted inp_to_out staircase matmul into the same PSUM tiles as
    the gen part.

The device does all floating-point work: both matmuls, softmaxes, both
scatters, gating and the final combine. Host work is layout only
(transposes, sorted slicing/padding, integer rank relabel) plus the
final transpose-gather of the output.
"""

import math

import numpy as np

import concourse.bass as bass
import concourse.mybir as mybir
import concourse.tile as tile
from concourse.bass_utils import run_bass_kernel_spmd

F32 = mybir.dt.float32

# -------------------- problem dims (hardcoded per spec) --------------------
FULL_CFG = dict(B=512, D=1024, S=2048, GEN_V=30000, INP_V=20000, OUT_V=50257)
NCORES = 8


def _derive(cfg):
    d = dict(cfg)
    d["VS"] = int(math.ceil(cfg["OUT_V"] / NCORES / 128)) * 128  # v-slice/core
    d["NT"] = d["VS"] // 128  # v-tiles per core
    d["BSH"] = cfg["B"] // NCORES  # batch rows per core (ptr part)
    d["RTOT"] = (cfg["INP_V"] + 127) // 128  # hist hi-chunks (global)
    d["SC"] = cfg["S"] // 128  # score chunks
    d["DC"] = cfg["D"] // 128  # feature chunks
    return d


# ============================ host-side planning ============================


def make_plan(cfg, gen_to_out, inp_to_out):
    """Static plan: sorted tables, per-core shard windows, staircase bases."""
    c = _derive(cfg)
    VS, NT = c["VS"], c["NT"]
    plan = {"cfg": c}

    # ---- gen table sort ----
    pg = np.argsort(gen_to_out, kind="stable")
    g2o_s = gen_to_out[pg].astype(np.int64)
    plan["gen_perm"] = pg
    cuts = np.array([k * VS for k in range(NCORES + 1)], dtype=np.int64)
    j0 = np.searchsorted(g2o_s, cuts[:-1], side="left")
    j1 = np.searchsorted(g2o_s, cuts[1:], side="left")
    gen_ch_tot = (len(g2o_s) + 127) // 128
    c0 = j0 // 128
    spans = (j1 + 127) // 128 - c0
    JCH = max(int(spans.max()), 1)
    c0 = np.minimum(c0, max(gen_ch_tot - JCH, 0))
    plan["JCH"], plan["gen_c0"], plan["gen_j0"], plan["gen_j1"] = JCH, c0, j0, j1

    # local (per-core) sorted tables for staircase bases
    g2o_loc = np.full((NCORES, JCH * 128), np.inf, dtype=np.float64)
    for k in range(NCORES):
        lo = c0[k] * 128
        hi = min(lo + JCH * 128, len(g2o_s))
        g2o_loc[k, : hi - lo] = g2o_s[lo:hi] - cuts[k]
    base_g, wg_w = _staircase_base(g2o_loc, NT, JCH)
    plan["base_g"], plan["W_G"] = base_g, wg_w

    # ---- ptr table sort ----
    pp = np.argsort(inp_to_out, kind="stable")
    ip2o_s = inp_to_out[pp].astype(np.int64)
    rank = np.empty_like(pp)
    rank[pp] = np.arange(len(pp))
    plan["rank"] = rank
    r0 = np.searchsorted(ip2o_s, cuts[:-1], side="left")
    r1 = np.searchsorted(ip2o_s, cuts[1:], side="left")
    rc0 = r0 // 128
    rspans = (r1 + 127) // 128 - rc0
    RCH = max(int(rspans.max()), 1)
    rc0 = np.minimum(rc0, max(c["RTOT"] - RCH, 0))
    plan["RCH"], plan["ptr_rc0"] = RCH, rc0
    ip2o_loc = np.full((NCORES, RCH * 128), np.inf, dtype=np.float64)
    for k in range(NCORES):
        lo = rc0[k] * 128
        hi = min(lo + RCH * 128, len(ip2o_s))
        ip2o_loc[k, : hi - lo] = ip2o_s[lo:hi] - cuts[k]
    base_p, wp_w = _staircase_base(ip2o_loc, NT, RCH)
    plan["base_p"], plan["W_P"] = base_p, wp_w

    plan["ip2o_s"] = ip2o_s
    plan["g2o_s"] = g2o_s
    plan["cuts"] = cuts
    return plan


def _staircase_base(tbl_loc, NT, NCH):
    """Shared per-tile base chunk + window width covering all cores.

    tbl_loc: (NCORES, NCH*128) ascending local v values (inf = pad)."""
    base = np.zeros(NT, dtype=np.int64)
    need_hi = np.zeros(NT, dtype=np.int64)
    any_t = np.zeros(NT, dtype=bool)
    clo_all = np.full((NCORES, NT), NCH, dtype=np.int64)
    chi_all = np.full((NCORES, NT), -1, dtype=np.int64)
    for k in range(NCORES):
        a = tbl_loc[k]
        lo = np.searchsorted(a, [t * 128 for t in range(NT)], side="left")
        hi = np.searchsorted(a, [(t + 1) * 128 for t in range(NT)], side="left")
        for t in range(NT):
            if hi[t] > lo[t]:
                clo_all[k, t] = lo[t] // 128
                chi_all[k, t] = (hi[t] - 1) // 128
                any_t[t] = True
    for t in range(NT):
        if any_t[t]:
            base[t] = clo_all[:, t].min()
            need_hi[t] = chi_all[:, t].max()
        else:
            base[t] = 0
            need_hi[t] = 0
    W = max(int((need_hi - base).max()) + 1, 1)
    base = np.minimum(base, NCH - W)
    base = np.maximum(base, 0)
    # verify coverage
    for t in range(NT):
        assert base[t] <= clo_all[:, t].min() or not any_t[t]
        assert need_hi[t] <= base[t] + W - 1
    return base, W


# ============================ device module ============================


def build_module(plan, dtype_build=F32):
    c = plan["cfg"]
    B, D, S = c["B"], c["D"], c["S"]
    VS, NT, BSH = c["VS"], c["NT"], c["BSH"]
    RTOT, SC, DC = c["RTOT"], c["SC"], c["DC"]
    JCH, RCH = plan["JCH"], plan["RCH"]
    W_G, W_P = plan["W_G"], plan["W_P"]
    base_g, base_p = plan["base_g"], plan["base_p"]
    rc0 = plan["ptr_rc0"]
    groups = [list(range(NCORES))]

    nc = bass.Bass()
    dp = nc.declare_dram_parameter
    x_T = dp("x_T", [D, B], F32, isOutput=False)
    x_T64 = dp("x_T64", [D, BSH], F32, isOutput=False)
    wp_ch = dp("wp_ch", [128, DC], F32, isOutput=False)
    bp_t = dp("bp", [1, 1], F32, isOutput=False)
    wg = dp("wg", [D, JCH * 128], F32, isOutput=False)
    bg_c = dp("bg_ch", [128, JCH], F32, isOutput=False)
    g2o = dp("g2o", [128, JCH], F32, isOutput=False)
    ip2o = dp("ip2o", [128, RCH], F32, isOutput=False)
    sco_T = dp("sco_T", [S, BSH], F32, isOutput=False)
    clo_T = dp("clo_T", [S, BSH], F32, isOutput=False)
    chi_T = dp("chi_T", [S, BSH], F32, isOutput=False)
    out_T = dp("out_T", [VS, B], F32, isOutput=True)

    AF = mybir.ActivationFunctionType
    OP = mybir.AluOpType

    with tile.TileContext(nc) as tc:
        with (
            tc.tile_pool(name="const", bufs=1) as cpool,
            tc.tile_pool(name="dram", bufs=1, space="DRAM") as dram,
        ):
            zin = dram.tile([1, B], F32, tag="zin")
            zout = dram.tile([1, B], F32, tag="zout")
            a2a_in = dram.tile([NCORES, RCH * 128, BSH], F32, tag="a2a_in")
            a2a_out = dram.tile([NCORES, RCH * 128, BSH], F32, tag="a2a_out")

            # ---- constants (consolidated tiles; 4KB/partition granularity) ----
            consts = cpool.tile([128, 128 + RTOT + 1], F32)
            iota128 = consts[:, 0:128]
            iotaR = consts[:, 128 : 128 + RTOT]
            ones_col = consts[:, 128 + RTOT : 128 + RTOT + 1]
            nc.gpsimd.iota(
                iota128, pattern=[[1, 128]], channel_multiplier=0,
                allow_small_or_imprecise_dtypes=True,
            )
            nc.gpsimd.iota(
                iotaR, pattern=[[1, RTOT]], channel_multiplier=0,
                allow_small_or_imprecise_dtypes=True,
            )
            nc.vector.memset(ones_col, 1.0)

            rows = cpool.tile([1, 3 * B + 2 * BSH + 128], F32)
            ones128 = rows[:, 3 * B + 2 * BSH : 3 * B + 2 * BSH + 128]
            nc.vector.memset(ones128, 1.0)
            interp_row = rows[:, 0:B]
            zpart = rows[:, B : 2 * B]
            zrow = rows[:, 2 * B : 3 * B]
            interp64 = rows[:, 3 * B : 3 * B + BSH]
            # scale rows computed later reuse zpart/interp slots via new names
            E_all = cpool.tile([128, JCH * B], F32)
            # H_all allocated in the Q-phase scope below (after HT frees)
            tbls = cpool.tile([128, 2 * JCH + RCH], F32)
            g2o_sb = tbls[:, 0:JCH]
            nc.sync.dma_start(out=g2o_sb, in_=g2o[:])
            ip2o_sb = tbls[:, JCH : JCH + RCH]
            nc.sync.dma_start(out=ip2o_sb, in_=ip2o[:])
            bg_sb = tbls[:, JCH + RCH : 2 * JCH + RCH]
            nc.sync.dma_start(out=bg_sb, in_=bg_c[:])

            with tc.tile_pool(name="xpool", bufs=1) as xpool:
                # ---- x_T, Wp, interp ----
                xts = xpool.tile([128, DC * B], F32)
                nc.sync.dma_start(
                    out=xts[:].rearrange("p (dc b) -> p dc b", b=B),
                    in_=x_T[:].rearrange("(dc p) b -> p dc b", p=128),
                )
                xts64 = xpool.tile([128, DC * BSH], F32)
                nc.sync.dma_start(
                    out=xts64[:].rearrange("p (dc b) -> p dc b", b=BSH),
                    in_=x_T64[:].rearrange("(dc p) b -> p dc b", p=128),
                )
                wp_sb = xpool.tile([128, DC], F32)
                nc.sync.dma_start(out=wp_sb[:], in_=wp_ch[:])
                bp_sb = xpool.tile([1, 1], F32)
                nc.sync.dma_start(out=bp_sb[:], in_=bp_t[:])

                with tc.tile_pool(name="ppsum", bufs=1, space="PSUM") as ppsum:
                    psum_i = ppsum.tile([1, B], F32)
                    for dc in range(DC):
                        nc.tensor.matmul(
                            psum_i[:],
                            lhsT=wp_sb[:, dc : dc + 1],
                            rhs=xts[:, dc * B : (dc + 1) * B],
                            start=(dc == 0),
                            stop=(dc == DC - 1),
                        )
                    nc.scalar.activation(
                        interp_row, psum_i[:], AF.Sigmoid, bias=bp_sb[:, :]
                    )
                    psum_i64 = ppsum.tile([1, BSH], F32)
                    for dc in range(DC):
                        nc.tensor.matmul(
                            psum_i64[:],
                            lhsT=wp_sb[:, dc : dc + 1],
                            rhs=xts64[:, dc * BSH : (dc + 1) * BSH],
                            start=(dc == 0),
                            stop=(dc == DC - 1),
                        )
                    nc.scalar.activation(
                        interp64, psum_i64[:], AF.Sigmoid, bias=bp_sb[:, :]
                    )

                # ---- phase 1: logits -> E, partial Z ----
                with (
                    tc.tile_pool(name="wgp", bufs=3) as wgp,
                    tc.tile_pool(name="lpsum", bufs=2, space="PSUM") as lpsum,
                    tc.tile_pool(name="zpsum", bufs=1, space="PSUM") as zpsum,
                ):
                    psum_z = zpsum.tile([1, B], F32)
                    for ci in range(JCH):
                        wgt = wgp.tile([128, DC * 128], F32)
                        nc.sync.dma_start(
                            out=wgt[:].rearrange("p (dc j) -> p dc j", j=128),
                            in_=wg[:, ci * 128 : (ci + 1) * 128].rearrange(
                                "(dc p) j -> p dc j", p=128
                            ),
                        )
                        psum_l = lpsum.tile([128, B], F32)
                        for dc in range(DC):
                            nc.tensor.matmul(
                                psum_l[:],
                                lhsT=wgt[:, dc * 128 : (dc + 1) * 128],
                                rhs=xts[:, dc * B : (dc + 1) * B],
                                start=(dc == 0),
                                stop=(dc == DC - 1),
                            )
                        nc.scalar.activation(
                            E_all[:, ci * B : (ci + 1) * B],
                            psum_l[:],
                            AF.Exp,
                            bias=bg_sb[:, ci : ci + 1],
                        )
                        nc.tensor.matmul(
                            psum_z[:],
                            lhsT=ones_col,
                            rhs=E_all[:, ci * B : (ci + 1) * B],
                            start=(ci == 0),
                            stop=(ci == JCH - 1),
                        )
                    nc.vector.tensor_copy(zpart, psum_z[:])

            # Z all-reduce (emit early; overlaps with ptr histogram below)
            nc.gpsimd.dma_start(zin[:], zpart)
            nc.gpsimd.collective_compute(
                "AllReduce",
                mybir.AluOpType.add,
                replica_groups=groups,
                ins=[zin.opt()],
                outs=[zout.opt()],
            )
            nc.gpsimd.dma_start(zrow, zout[:])

            with (
                tc.tile_pool(name="hist", bufs=1) as hist,
                tc.tile_pool(name="wbuild", bufs=2) as wbuild,
                tc.tile_pool(name="ubuild", bufs=2) as ubuild,
                tc.tile_pool(name="hpsum", bufs=2, space="PSUM") as hpsum,
                tc.tile_pool(name="zspsum", bufs=1, space="PSUM") as zspsum,
            ):
                # ---- ptr: alphas ----
                sco_sb = hist.tile([128, SC * BSH], F32)
                nc.sync.dma_start(
                    out=sco_sb[:].rearrange("p (sc b) -> p sc b", b=BSH),
                    in_=sco_T[:].rearrange("(sc p) b -> p sc b", p=128),
                )
                clo_sb = hist.tile([128, SC * BSH], F32)
                nc.sync.dma_start(
                    out=clo_sb[:].rearrange("p (sc b) -> p sc b", b=BSH),
                    in_=clo_T[:].rearrange("(sc p) b -> p sc b", p=128),
                )
                chi_sb = hist.tile([128, SC * BSH], F32)
                nc.sync.dma_start(
                    out=chi_sb[:].rearrange("p (sc b) -> p sc b", b=BSH),
                    in_=chi_T[:].rearrange("(sc p) b -> p sc b", p=128),
                )
                A_all = hist.tile([128, SC * BSH], F32)
                nc.scalar.activation(A_all[:], sco_sb[:], AF.Exp)
                hrows = hist.tile([1, 4 * BSH], F32)
                zs_inv = hrows[:, 0:BSH]
                cs_row = hrows[:, BSH : 2 * BSH]
                psum_zs = zspsum.tile([1, BSH], F32)
                for sc in range(SC):
                    nc.tensor.matmul(
                        psum_zs[:],
                        lhsT=ones_col,
                        rhs=A_all[:, sc * BSH : (sc + 1) * BSH],
                        start=(sc == 0),
                        stop=(sc == SC - 1),
                    )
                zs_sb = hrows[:, 2 * BSH : 3 * BSH]
                nc.vector.tensor_copy(zs_sb, psum_zs[:])
                nc.vector.reciprocal(zs_inv, zs_sb)
                om_row = hrows[:, 3 * BSH : 4 * BSH]
                nc.vector.tensor_scalar(
                    om_row, interp64, -1.0, 1.0, OP.mult, OP.add
                )
                nc.vector.tensor_tensor(cs_row, om_row, zs_inv, OP.mult)
                cs_bc = hist.tile([128, BSH], F32)
                psum_bc = zspsum.tile([128, BSH], F32, tag="bc")
                nc.tensor.matmul(
                    psum_bc[:], lhsT=ones128, rhs=cs_row, start=True, stop=True
                )
                nc.scalar.activation(cs_bc[:], psum_bc[:], AF.Copy)
                for sc in range(SC):
                    sl = slice(sc * BSH, (sc + 1) * BSH)
                    nc.vector.tensor_tensor(
                        A_all[:, sl], A_all[:, sl], cs_bc[:], OP.mult
                    )

                # ---- ptr: per-row histogram into PSUM ----
                HT = hist.tile([128, RTOT * BSH], F32)
                for b in range(BSH):
                    Wt = wbuild.tile([128, SC * 128], dtype_build)
                    lo_b = (
                        clo_sb[:]
                        .rearrange("p (sc b) -> p sc b", b=BSH)[:, :, b : b + 1]
                        .to_broadcast([128, SC, 128])
                    )
                    nc.vector.tensor_tensor(
                        Wt[:].rearrange("p (sc j) -> p sc j", j=128),
                        lo_b,
                        iota128.rearrange("p (o j) -> p o j", o=1).to_broadcast([128, SC, 128]),
                        OP.is_equal,
                    )
                    a_b = (
                        A_all[:]
                        .rearrange("p (sc b) -> p sc b", b=BSH)[:, :, b : b + 1]
                        .to_broadcast([128, SC, 128])
                    )
                    nc.vector.tensor_tensor(
                        Wt[:].rearrange("p (sc j) -> p sc j", j=128),
                        Wt[:].rearrange("p (sc j) -> p sc j", j=128),
                        a_b,
                        OP.mult,
                    )
                    Ut = ubuild.tile([128, SC * RTOT], dtype_build)
                    hi_b = (
                        chi_sb[:]
                        .rearrange("p (sc b) -> p sc b", b=BSH)[:, :, b : b + 1]
                        .to_broadcast([128, SC, RTOT])
                    )
                    nc.vector.tensor_tensor(
                        Ut[:].rearrange("p (sc j) -> p sc j", j=RTOT),
                        hi_b,
                        iotaR.rearrange("p (o j) -> p o j", o=1).to_broadcast([128, SC, RTOT]),
                        OP.is_equal,
                    )
                    psum_h = hpsum.tile([128, RTOT], F32)
                    for sc in range(SC):
                        nc.tensor.matmul(
                            psum_h[:],
                            lhsT=Wt[:, sc * 128 : (sc + 1) * 128],
                            rhs=Ut[:, sc * RTOT : (sc + 1) * RTOT],
                            start=(sc == 0),
                            stop=(sc == SC - 1),
                        )
                    nc.vector.tensor_copy(
                        HT[:].rearrange("p (c b) -> p c b", b=BSH)[:, :, b],
                        psum_h[:],
                    )

                # ---- hist exchange (AllToAll) ----
                for p in range(NCORES):
                    nc.sync.dma_start(
                        out=a2a_in[:][p].rearrange("(j lo) b -> lo j b", lo=128),
                        in_=HT[:].rearrange("p (c b) -> p c b", b=BSH)[
                            :, rc0[p] : rc0[p] + RCH, :
                        ],
                    )
                nc.gpsimd.collective_compute(
                    "AllToAll",
                    mybir.AluOpType.bypass,
                    replica_groups=groups,
                    ins=[a2a_in.opt()],
                    outs=[a2a_out.opt()],
                )

            # ---- E scale by interp/Z ----
            zinv = rows[:, B : 2 * B]  # reuse zpart slot (already consumed)
            nc.vector.reciprocal(zinv, zrow)
            scale_row = rows[:, 0:B]  # in-place over interp_row (last use)
            nc.vector.tensor_tensor(scale_row, interp_row, zinv, OP.mult)
            scale_bc = cpool.tile([128, B], F32)
            with tc.tile_pool(name="bcps", bufs=1, space="PSUM") as bcps:
                psum_sc = bcps.tile([128, B], F32)
                nc.tensor.matmul(
                    psum_sc[:], lhsT=ones128, rhs=scale_row, start=True, stop=True
                )
                nc.scalar.activation(scale_bc[:], psum_sc[:], AF.Copy)
            for ci in range(JCH):
                sl = slice(ci * B, (ci + 1) * B)
                nc.vector.tensor_tensor(
                    E_all[:, sl], E_all[:, sl], scale_bc[:], OP.mult
                )

            # ---- load exchanged hist: H_all[lo, j*B + p*BSH + b] ----
            hq_cm = tc.tile_pool(name="hq", bufs=1)
            hq = hq_cm.__enter__()
            H_all = hq.tile([128, RCH * B], F32)
            for j in range(RCH):
                nc.sync.dma_start(
                    out=H_all[:, j * B : (j + 1) * B].rearrange(
                        "p (pp b) -> p pp b", b=BSH
                    ),
                    in_=a2a_out[:].rearrange("pp (j lo) b -> lo pp j b", lo=128)[
                        :, :, j, :
                    ],
                )

            # ---- staircase Q matmuls into output tiles ----
            with (
                tc.tile_pool(name="qpool", bufs=3) as qpool,
                tc.tile_pool(name="opool", bufs=3) as opool,
                tc.tile_pool(name="opsum", bufs=2, space="PSUM") as opsum,
            ):
                for t in range(NT):
                    psum_o = opsum.tile([128, B], F32)
                    for w in range(W_G):
                        ci = int(base_g[t]) + w
                        Qt = qpool.tile([128, 128], F32)
                        nc.vector.tensor_scalar(
                            Qt[:],
                            iota128,
                            g2o_sb[:, ci : ci + 1],
                            float(-t * 128),
                            OP.subtract,
                            OP.is_equal,
                        )
                        nc.tensor.matmul(
                            psum_o[:],
                            lhsT=Qt[:],
                            rhs=E_all[:, ci * B : (ci + 1) * B],
                            start=(w == 0),
                            stop=False,
                        )
                    for w in range(W_P):
                        ci = int(base_p[t]) + w
                        Qt = qpool.tile([128, 128], F32)
                        nc.vector.tensor_scalar(
                            Qt[:],
                            iota128,
                            ip2o_sb[:, ci : ci + 1],
                            float(-t * 128),
                            OP.subtract,
                            OP.is_equal,
                        )
                        nc.tensor.matmul(
                            psum_o[:],
                            lhsT=Qt[:],
                            rhs=H_all[:, ci * B : (ci + 1) * B],
                            start=False,
                            stop=(w == W_P - 1),
                        )
                    ot = opool.tile([128, B], F32)
                    nc.scalar.activation(ot[:], psum_o[:], AF.Copy)
                    nc.sync.dma_start(
                        out=out_T[t * 128 : (t + 1) * 128, :], in_=ot[:]
                    )
            hq_cm.__exit__(None, None, None)
    return nc


def split_excess_waits(nc, limit=1):
    """Walrus codegen rejects instructions carrying several sem waits.
    Move excess waits onto preceding no-ops on the same engine."""
    n = 0
    for f in nc.m.functions:
        for bb in f.blocks:
            out = []
            for ins in bb.instructions:
                si = ins.sync_info
                if si is not None and si.on_wait and len(si.on_wait) > limit:
                    waits = list(si.on_wait)
                    for i in range(0, len(waits), limit):
                        nop = mybir.InstNoOp(
                            name=f"I-wsplit-{n}",
                            engine=ins.engine,
                            bass_nofuse=True,
                            sync_info=mybir.SyncInfo(
                                on_wait=waits[i : i + limit], on_update=[]
                            ),
                        )
                        n += 1
                        out.append(nop)
                    si.on_wait.clear()
                out.append(ins)
            bb.instructions[:] = out
    return n


# ============================ host orchestration ============================


def make_in_maps(plan, x, scores, Wp, bp, Wg, bg, ctx_inp, gen_to_out, inp_to_out):
    c = plan["cfg"]
    B, D, S = c["B"], c["D"], c["S"]
    BSH, DC = c["BSH"], c["DC"]
    JCH, RCH = plan["JCH"], plan["RCH"]
    cuts = plan["cuts"]
    pg = plan["gen_perm"]
    g2o_s = plan["g2o_s"]
    ip2o_s = plan["ip2o_s"]
    rank = plan["rank"]

    x = np.asarray(x, dtype=np.float32)
    scores = np.asarray(scores, dtype=np.float32)
    Wg_s = np.asarray(Wg, dtype=np.float32)[:, pg]
    bg_s = np.asarray(bg, dtype=np.float32)[pg]

    x_T = np.ascontiguousarray(x.T)
    wp_ch = np.ascontiguousarray(
        np.asarray(Wp, dtype=np.float32).reshape(DC, 128).T
    )
    bp_t = np.asarray(bp, dtype=np.float32).reshape(1, 1)

    ctx_rank = rank[np.asarray(ctx_inp)]  # (B, S) int
    chi = (ctx_rank >> 7).astype(np.float32)
    clo = (ctx_rank & 127).astype(np.float32)

    in_maps = []
    for k in range(NCORES):
        m = {"x_T": x_T, "wp_ch": wp_ch, "bp": bp_t}
        bs = slice(k * BSH, (k + 1) * BSH)
        m["x_T64"] = np.ascontiguousarray(x[bs].T)
        m["sco_T"] = np.ascontiguousarray(scores[bs].T)
        m["clo_T"] = np.ascontiguousarray(clo[bs].T)
        m["chi_T"] = np.ascontiguousarray(chi[bs].T)

        # gen shard
        j0, j1 = plan["gen_j0"][k], plan["gen_j1"][k]
        lo = plan["gen_c0"][k] * 128
        wgk = np.zeros((D, JCH * 128), dtype=np.float32)
        bgk = np.full(JCH * 128, -1e9, dtype=np.float32)
        g2k = np.full((128, JCH), 1e9, dtype=np.float32)
        hi = min(lo + JCH * 128, len(g2o_s))
        wgk[:, : hi - lo] = Wg_s[:, lo:hi]
        # bias: valid only inside [j0, j1)
        va, vb = max(j0 - lo, 0), max(j1 - lo, 0)
        bgk[va:vb] = bg_s[lo + va : lo + vb]
        bgk = np.ascontiguousarray(bgk.reshape(JCH, 128).T)
        tmp = np.full(JCH * 128, 1e9, dtype=np.float32)
        tmp[: hi - lo] = (g2o_s[lo:hi] - cuts[k]).astype(np.float32)
        g2k = np.ascontiguousarray(tmp.reshape(JCH, 128).T)
        m["wg"], m["bg_ch"], m["g2o"] = wgk, bgk, g2k

        # ptr shard table
        rlo = plan["ptr_rc0"][k] * 128
        rhi = min(rlo + RCH * 128, len(ip2o_s))
        tmp = np.full(RCH * 128, 1e9, dtype=np.float32)
        tmp[: rhi - rlo] = (ip2o_s[rlo:rhi] - cuts[k]).astype(np.float32)
        m["ip2o"] = np.ascontiguousarray(tmp.reshape(RCH, 128).T)
        in_maps.append(m)
    return in_maps


def assemble_output(plan, results):
    c = plan["cfg"]
    B, OUT_V, VS = c["B"], c["OUT_V"], c["VS"]
    out = np.empty((B, OUT_V), dtype=np.float32)
    for k in range(NCORES):
        v0 = k * VS
        n = min(VS, OUT_V - v0)
        if n <= 0:
            break
        out[:, v0 : v0 + n] = results[k]["out_T"][:n].T
    return out


_CACHE = {}


def kernel(x, scores, Wp, bp, Wg, bg, ctx_inp, gen_to_out, inp_to_out):
    gen_to_out = np.asarray(gen_to_out)
    inp_to_out = np.asarray(inp_to_out)
    plan = make_plan(FULL_CFG, gen_to_out, inp_to_out)
    key = (plan["JCH"], plan["RCH"], plan["W_G"], plan["W_P"],
           tuple(plan["base_g"]), tuple(plan["base_p"]))
    if key not in _CACHE:
        nc = build_module(plan)
        split_excess_waits(nc)
        _CACHE[key] = nc
    nc = _CACHE[key]
    in_maps = make_in_maps(
        plan, x, scores, Wp, bp, Wg, bg, ctx_inp, gen_to_out, inp_to_out
    )
    res = run_bass_kernel_spmd(nc, in_maps, list(range(NCORES)))
    return assemble_output(plan, res.results)


if __name__ == "__main__":
    import reference

    inputs = reference.setup_inputs()
    inputs = {k: np.asarray(v) for k, v in inputs.items()}
    out = kernel(**inputs)
    exp = np.asarray(reference.reference(**{k: v for k, v in inputs.items()}))
    err = np.abs(out - exp).max() / (np.abs(exp).max() + 1e-30)
    print("rel err:", err)


# revision 22
# speedup vs baseline: 1.5154x; 1.5154x over previous
"""Pointer-generator output layer on 8 Trainium2 NeuronCores (Bass/Tile).

Algorithm (per reference.py):
  interp = sigmoid(x @ Wp + bp)                              (B, 1)
  gen    = softmax(x @ Wg + bg)  scattered via gen_to_out    (B, OUT_V)
  ptr    = softmax(scores)       scattered via inp_to_out[ctx_inp]
  out    = interp * gen + (1 - interp) * ptr

Sharding: output-vocab sharded across 8 cores. Host sorts the (static,
shared) index tables so both scatters become monotone "staircase" 0/1
matmuls on device:
  - Wg columns permuted by argsort(gen_to_out): each core reads only the
    Wg columns mapping into its vocab slice; softmax stats all-reduced.
  - inp_to_out sorted; ctx_inp relabeled by rank (host int gather). On
    device a per-row histogram over the 20000 rank bins is built with
    one-hot matmuls accumulating in PSUM (exact fp32), batch-sharded;
    the per-rank histograms are exchanged (AllToAll) and applied through
    the sorted inp_to_out staircase matmul into the same PSUM tiles as
    the gen part.

The device does all floating-point work: both matmuls, softmaxes, both
scatters, gating and the final combine. Host work is layout only
(transposes, sorted slicing/padding, integer rank relabel) plus the
final transpose-gather of the output.
"""

import math

import numpy as np

import concourse.bass as bass
import concourse.mybir as mybir
import concourse.tile as tile
from concourse.bass_utils import run_bass_kernel_spmd

F32 = mybir.dt.float32

# -------------------- problem dims (hardcoded per spec) --------------------
FULL_CFG = dict(B=512, D=1024, S=2048, GEN_V=30000, INP_V=20000, OUT_V=50257)
NCORES = 8


def _derive(cfg):
    d = dict(cfg)
    d["VS"] = int(math.ceil(cfg["OUT_V"] / NCORES / 128)) * 128  # v-slice/core
    d["NT"] = d["VS"] // 128  # v-tiles per core
    d["BSH"] = cfg["B"] // NCORES  # batch rows per core (ptr part)
    d["RTOT"] = (cfg["INP_V"] + 127) // 128  # hist hi-chunks (global)
    d["SC"] = cfg["S"] // 128  # score chunks
    d["DC"] = cfg["D"] // 128  # feature chunks
    return d


# ============================ host-side planning ============================


def make_plan(cfg, gen_to_out, inp_to_out):
    """Static plan: sorted tables, per-core shard windows, staircase bases."""
    c = _derive(cfg)
    VS, NT = c["VS"], c["NT"]
    plan = {"cfg": c}

    # ---- gen table sort ----
    pg = np.argsort(gen_to_out, kind="stable")
    g2o_s = gen_to_out[pg].astype(np.int64)
    plan["gen_perm"] = pg
    cuts = np.array([k * VS for k in range(NCORES + 1)], dtype=np.int64)
    j0 = np.searchsorted(g2o_s, cuts[:-1], side="left")
    j1 = np.searchsorted(g2o_s, cuts[1:], side="left")
    c0 = j0 // 128
    spans = (j1 + 127) // 128 - c0
    JCH = max(int(spans.max()), 1)
    plan["JCH"], plan["gen_c0"], plan["gen_j0"], plan["gen_j1"] = JCH, c0, j0, j1

    # local (per-core) sorted tables for staircase bases
    g2o_loc = np.full((NCORES, JCH * 128), np.inf, dtype=np.float64)
    for k in range(NCORES):
        lo = c0[k] * 128
        hi = min(lo + JCH * 128, len(g2o_s))
        g2o_loc[k, : hi - lo] = g2o_s[lo:hi] - cuts[k]
    plan["wins_g"], plan["span_g"] = _staircase_base(g2o_loc, NT, JCH)

    # ---- ptr table sort ----
    pp = np.argsort(inp_to_out, kind="stable")
    ip2o_s = inp_to_out[pp].astype(np.int64)
    rank = np.empty_like(pp)
    rank[pp] = np.arange(len(pp))
    plan["rank"] = rank
    r0 = np.searchsorted(ip2o_s, cuts[:-1], side="left")
    r1 = np.searchsorted(ip2o_s, cuts[1:], side="left")
    rc0 = r0 // 128
    rspans = (r1 + 127) // 128 - rc0
    RCH = max(int(rspans.max()), 1)
    plan["RCH"], plan["ptr_rc0"] = RCH, rc0
    plan["RTOT_PAD"] = max(int(rc0.max()) + RCH, c["RTOT"])
    ip2o_loc = np.full((NCORES, RCH * 128), np.inf, dtype=np.float64)
    for k in range(NCORES):
        lo = rc0[k] * 128
        hi = min(lo + RCH * 128, len(ip2o_s))
        ip2o_loc[k, : hi - lo] = ip2o_s[lo:hi] - cuts[k]
    plan["wins_p"], plan["span_p"] = _staircase_base(ip2o_loc, NT, RCH)

    plan["ip2o_s"] = ip2o_s
    plan["g2o_s"] = g2o_s
    plan["cuts"] = cuts
    return plan


def _staircase_base(tbl_loc, NT, NCH):
    """Shared per-tile base chunk + window width covering all cores.

    tbl_loc: (NCORES, NCH*128) ascending local v values (inf = pad)."""
    base = np.zeros(NT, dtype=np.int64)
    need_hi = np.zeros(NT, dtype=np.int64)
    any_t = np.zeros(NT, dtype=bool)
    clo_all = np.full((NCORES, NT), NCH, dtype=np.int64)
    chi_all = np.full((NCORES, NT), -1, dtype=np.int64)
    for k in range(NCORES):
        a = tbl_loc[k]
        lo = np.searchsorted(a, [t * 128 for t in range(NT)], side="left")
        hi = np.searchsorted(a, [(t + 1) * 128 for t in range(NT)], side="left")
        for t in range(NT):
            if hi[t] > lo[t]:
                clo_all[k, t] = lo[t] // 128
                chi_all[k, t] = (hi[t] - 1) // 128
                any_t[t] = True
    # per-tile union window over cores: [base[t], need_hi[t]] inclusive
    for t in range(NT):
        if any_t[t]:
            base[t] = clo_all[:, t].min()
            need_hi[t] = chi_all[:, t].max()
        else:
            base[t] = 0
            need_hi[t] = -1  # empty window -> no MMs for this tile
    wins = [
        (int(base[t]), int(need_hi[t]) - int(base[t]) + 1) for t in range(NT)
    ]
    # tile span per chunk (for one-shot Q prebuild): chunk ci is used by
    # tiles t in [tmin[ci], tmax[ci]]
    tmin = {}
    tmax = {}
    for t, (b0, w) in enumerate(wins):
        for ci in range(b0, b0 + w):
            tmin[ci] = min(tmin.get(ci, t), t)
            tmax[ci] = max(tmax.get(ci, t), t)
    return wins, (tmin, tmax)


# ============================ device module ============================


def build_module(plan, dtype_build=mybir.dt.bfloat16):
    c = plan["cfg"]
    B, D, S = c["B"], c["D"], c["S"]
    VS, NT, BSH = c["VS"], c["NT"], c["BSH"]
    RTOT, SC, DC = c["RTOT"], c["SC"], c["DC"]
    RTP = plan["RTOT_PAD"]
    JCH, RCH = plan["JCH"], plan["RCH"]
    wins_g, (g_tmin, g_tmax) = plan["wins_g"], plan["span_g"]
    wins_p, (p_tmin, p_tmax) = plan["wins_p"], plan["span_p"]
    rc0 = plan["ptr_rc0"]
    groups = [list(range(NCORES))]
    DTB = dtype_build
    # widest per-chunk Q prebuild (in tiles of 128)
    gq_w = max(g_tmax[ci] - g_tmin[ci] + 1 for ci in g_tmin) if g_tmin else 1
    pq_w = max(p_tmax[ci] - p_tmin[ci] + 1 for ci in p_tmin) if p_tmin else 1
    QW = max(gq_w, pq_w)

    nc = bass.Bass()
    dp = nc.declare_dram_parameter
    x_T = dp("x_T", [D, B], F32, isOutput=False)
    x_T64 = dp("x_T64", [D, BSH], F32, isOutput=False)
    wp_ch = dp("wp_ch", [128, DC], F32, isOutput=False)
    bp_t = dp("bp", [1, 1], F32, isOutput=False)
    wg = dp("wg", [D, JCH * 128], F32, isOutput=False)
    bg_c = dp("bg_ch", [128, JCH], F32, isOutput=False)
    g2o = dp("g2o", [128, JCH], F32, isOutput=False)
    ip2o = dp("ip2o", [128, RCH], F32, isOutput=False)
    sco_T = dp("sco_T", [S, BSH], F32, isOutput=False)
    clo_T = dp("clo_T", [S, BSH], DTB, isOutput=False)
    chi_T = dp("chi_T", [S, BSH], DTB, isOutput=False)
    out_T = dp("out_T", [VS, B], F32, isOutput=True)

    AF = mybir.ActivationFunctionType
    OP = mybir.AluOpType

    with tile.TileContext(nc) as tc:
        with (
            tc.tile_pool(name="const", bufs=1) as cpool,
            tc.tile_pool(name="dram", bufs=1, space="DRAM") as dram,
        ):
            zin = dram.tile([1, B], F32, tag="zin")
            zout = dram.tile([1, B], F32, tag="zout")
            a2a_in = dram.tile([NCORES, RCH * 128, BSH], F32, tag="a2a_in")
            a2a_out = dram.tile([NCORES, RCH * 128, BSH], F32, tag="a2a_out")

            # ---- constants ----
            consts = cpool.tile([128, 128 + RTOT + 1], F32)
            iota128 = consts[:, 0:128]
            iotaR = consts[:, 128 : 128 + RTOT]
            ones_col = consts[:, 128 + RTOT : 128 + RTOT + 1]
            nc.gpsimd.iota(
                iota128, pattern=[[1, 128]], channel_multiplier=0,
                allow_small_or_imprecise_dtypes=True,
            )
            nc.gpsimd.iota(
                iotaR, pattern=[[1, RTOT]], channel_multiplier=0,
                allow_small_or_imprecise_dtypes=True,
            )
            nc.vector.memset(ones_col, 1.0)
            constb = cpool.tile([128, 128 + RTOT], DTB)
            iota128b = constb[:, 0:128]
            iotaRb = constb[:, 128 : 128 + RTOT]
            nc.vector.tensor_copy(iota128b, iota128)
            nc.vector.tensor_copy(iotaRb, iotaR)
            qiota = cpool.tile([128, QW * 128], F32)
            nc.gpsimd.iota(
                qiota[:], pattern=[[1, QW * 128]], channel_multiplier=0,
                allow_small_or_imprecise_dtypes=True,
            )

            rows = cpool.tile([1, 3 * B + 2 * BSH + 128], F32)
            ones128 = rows[:, 3 * B + 2 * BSH : 3 * B + 2 * BSH + 128]
            nc.vector.memset(ones128, 1.0)
            interp_row = rows[:, 0:B]
            zpart = rows[:, B : 2 * B]
            zrow = rows[:, 2 * B : 3 * B]
            interp64 = rows[:, 3 * B : 3 * B + BSH]
            E_all = cpool.tile([128, JCH * B], F32)
            tbls = cpool.tile([128, 2 * JCH + RCH], F32)
            g2o_sb = tbls[:, 0:JCH]
            nc.sync.dma_start(out=g2o_sb, in_=g2o[:])
            ip2o_sb = tbls[:, JCH : JCH + RCH]
            nc.sync.dma_start(out=ip2o_sb, in_=ip2o[:])
            bg_sb = tbls[:, JCH + RCH : 2 * JCH + RCH]
            nc.sync.dma_start(out=bg_sb, in_=bg_c[:])

            # ---- interp (needed by hist scale + gen scale) ----
            with tc.tile_pool(name="xw", bufs=1) as xw:
                wp_sb = xw.tile([128, DC], F32)
                nc.sync.dma_start(out=wp_sb[:], in_=wp_ch[:])
                bp_sb = xw.tile([1, 1], F32)
                nc.sync.dma_start(out=bp_sb[:], in_=bp_t[:])
                xts = cpool.tile([128, DC * B], F32)
                nc.sync.dma_start(
                    out=xts[:].rearrange("p (dc b) -> p dc b", b=B),
                    in_=x_T[:].rearrange("(dc p) b -> p dc b", p=128),
                )
                xts64 = xw.tile([128, DC * BSH], F32)
                nc.sync.dma_start(
                    out=xts64[:].rearrange("p (dc b) -> p dc b", b=BSH),
                    in_=x_T64[:].rearrange("(dc p) b -> p dc b", p=128),
                )
                with tc.tile_pool(name="ppsum", bufs=1, space="PSUM") as ppsum:
                    psum_i = ppsum.tile([1, B], F32)
                    for dc in range(DC):
                        nc.tensor.matmul(
                            psum_i[:],
                            lhsT=wp_sb[:, dc : dc + 1],
                            rhs=xts[:, dc * B : (dc + 1) * B],
                            start=(dc == 0),
                            stop=(dc == DC - 1),
                        )
                    nc.scalar.activation(
                        interp_row, psum_i[:], AF.Sigmoid, bias=bp_sb[:, :]
                    )
                    psum_i64 = ppsum.tile([1, BSH], F32)
                    for dc in range(DC):
                        nc.tensor.matmul(
                            psum_i64[:],
                            lhsT=wp_sb[:, dc : dc + 1],
                            rhs=xts64[:, dc * BSH : (dc + 1) * BSH],
                            start=(dc == 0),
                            stop=(dc == DC - 1),
                        )
                    nc.scalar.activation(
                        interp64, psum_i64[:], AF.Sigmoid, bias=bp_sb[:, :]
                    )

            # ---- ptr histogram first: its AllToAll overlaps the big matmul ----
            with (
                tc.tile_pool(name="hist", bufs=1) as hist,
                tc.tile_pool(name="wbuild", bufs=2) as wbuild,
                tc.tile_pool(name="ubuild", bufs=2) as ubuild,
                tc.tile_pool(name="hpsum", bufs=2, space="PSUM") as hpsum,
                tc.tile_pool(name="zspsum", bufs=1, space="PSUM") as zspsum,
            ):
                sco_sb = hist.tile([128, SC * BSH], F32)
                nc.sync.dma_start(
                    out=sco_sb[:].rearrange("p (sc b) -> p sc b", b=BSH),
                    in_=sco_T[:].rearrange("(sc p) b -> p sc b", p=128),
                )
                clo_sb = hist.tile([128, SC * BSH], DTB)
                nc.sync.dma_start(
                    out=clo_sb[:].rearrange("p (sc b) -> p sc b", b=BSH),
                    in_=clo_T[:].rearrange("(sc p) b -> p sc b", p=128),
                )
                chi_sb = hist.tile([128, SC * BSH], DTB)
                nc.sync.dma_start(
                    out=chi_sb[:].rearrange("p (sc b) -> p sc b", b=BSH),
                    in_=chi_T[:].rearrange("(sc p) b -> p sc b", p=128),
                )
                A_all = hist.tile([128, SC * BSH], F32)
                nc.scalar.activation(A_all[:], sco_sb[:], AF.Exp)
                hrows = hist.tile([1, 4 * BSH], F32)
                zs_inv = hrows[:, 0:BSH]
                cs_row = hrows[:, BSH : 2 * BSH]
                psum_zs = zspsum.tile([1, BSH], F32)
                for sc in range(SC):
                    nc.tensor.matmul(
                        psum_zs[:],
                        lhsT=ones_col,
                        rhs=A_all[:, sc * BSH : (sc + 1) * BSH],
                        start=(sc == 0),
                        stop=(sc == SC - 1),
                    )
                zs_sb = hrows[:, 2 * BSH : 3 * BSH]
                nc.vector.tensor_copy(zs_sb, psum_zs[:])
                nc.vector.reciprocal(zs_inv, zs_sb)
                om_row = hrows[:, 3 * BSH : 4 * BSH]
                nc.vector.tensor_scalar(
                    om_row, interp64, -1.0, 1.0, OP.mult, OP.add
                )
                nc.vector.tensor_tensor(cs_row, om_row, zs_inv, OP.mult)
                cs_bc = hist.tile([128, BSH], F32)
                psum_bc = zspsum.tile([128, BSH], F32, tag="bc")
                nc.tensor.matmul(
                    psum_bc[:], lhsT=ones128, rhs=cs_row, start=True, stop=True
                )
                nc.scalar.activation(cs_bc[:], psum_bc[:], AF.Copy)
                # A (bf16, alpha-scaled)
                Ab = hist.tile([128, SC * BSH], DTB)
                for sc in range(SC):
                    sl = slice(sc * BSH, (sc + 1) * BSH)
                    nc.vector.tensor_tensor(
                        Ab[:, sl], A_all[:, sl], cs_bc[:], OP.mult
                    )

                HT = hist.tile([128, RTP * BSH], F32)
                if RTP > RTOT:
                    nc.vector.memset(HT[:, RTOT * BSH : RTP * BSH], 0.0)
                for b in range(BSH):
                    Wt = wbuild.tile([128, SC * 128], DTB)
                    lo_b = (
                        clo_sb[:]
                        .rearrange("p (sc b) -> p sc b", b=BSH)[:, :, b : b + 1]
                        .to_broadcast([128, SC, 128])
                    )
                    nc.vector.tensor_tensor(
                        Wt[:].rearrange("p (sc j) -> p sc j", j=128),
                        lo_b,
                        iota128b.rearrange("p (o j) -> p o j", o=1).to_broadcast(
                            [128, SC, 128]
                        ),
                        OP.is_equal,
                    )
                    a_b = (
                        Ab[:]
                        .rearrange("p (sc b) -> p sc b", b=BSH)[:, :, b : b + 1]
                        .to_broadcast([128, SC, 128])
                    )
                    nc.vector.tensor_tensor(
                        Wt[:].rearrange("p (sc j) -> p sc j", j=128),
                        Wt[:].rearrange("p (sc j) -> p sc j", j=128),
                        a_b,
                        OP.mult,
                    )
                    Ut = ubuild.tile([128, SC * RTOT], DTB)
                    hi_b = (
                        chi_sb[:]
                        .rearrange("p (sc b) -> p sc b", b=BSH)[:, :, b : b + 1]
                        .to_broadcast([128, SC, RTOT])
                    )
                    nc.vector.tensor_tensor(
                        Ut[:].rearrange("p (sc j) -> p sc j", j=RTOT),
                        hi_b,
                        iotaRb.rearrange("p (o j) -> p o j", o=1).to_broadcast(
                            [128, SC, RTOT]
                        ),
                        OP.is_equal,
                    )
                    psum_h = hpsum.tile([128, RTOT], F32)
                    for sc in range(SC):
                        nc.tensor.matmul(
                            psum_h[:],
                            lhsT=Wt[:, sc * 128 : (sc + 1) * 128],
                            rhs=Ut[:, sc * RTOT : (sc + 1) * RTOT],
                            start=(sc == 0),
                            stop=(sc == SC - 1),
                        )
                    nc.vector.tensor_copy(
                        HT[:].rearrange("p (c b) -> p c b", b=BSH)[:, 0:RTOT, b],
                        psum_h[:],
                    )

                # ---- hist exchange (AllToAll) ----
                for p in range(NCORES):
                    nc.sync.dma_start(
                        out=a2a_in[:][p].rearrange("(j lo) b -> lo j b", lo=128),
                        in_=HT[:].rearrange("p (c b) -> p c b", b=BSH)[
                            :, rc0[p] : rc0[p] + RCH, :
                        ],
                    )
                nc.gpsimd.collective_compute(
                    "AllToAll",
                    mybir.AluOpType.bypass,
                    replica_groups=groups,
                    ins=[a2a_in.opt()],
                    outs=[a2a_out.opt()],
                )

            # ---- phase 1: logits -> E, partial Z (overlaps AllToAll) ----
            with (
                tc.tile_pool(name="wgp", bufs=3) as wgp,
                tc.tile_pool(name="lpsum", bufs=2, space="PSUM") as lpsum,
                tc.tile_pool(name="zpsum", bufs=1, space="PSUM") as zpsum,
            ):
                psum_z = zpsum.tile([1, B], F32)
                for ci in range(JCH):
                    wgt = wgp.tile([128, DC * 128], F32)
                    nc.sync.dma_start(
                        out=wgt[:].rearrange("p (dc j) -> p dc j", j=128),
                        in_=wg[:, ci * 128 : (ci + 1) * 128].rearrange(
                            "(dc p) j -> p dc j", p=128
                        ),
                    )
                    psum_l = lpsum.tile([128, B], F32)
                    for dc in range(DC):
                        nc.tensor.matmul(
                            psum_l[:],
                            lhsT=wgt[:, dc * 128 : (dc + 1) * 128],
                            rhs=xts[:, dc * B : (dc + 1) * B],
                            start=(dc == 0),
                            stop=(dc == DC - 1),
                        )
                    nc.scalar.activation(
                        E_all[:, ci * B : (ci + 1) * B],
                        psum_l[:],
                        AF.Exp,
                        bias=bg_sb[:, ci : ci + 1],
                    )
                    nc.tensor.matmul(
                        psum_z[:],
                        lhsT=ones_col,
                        rhs=E_all[:, ci * B : (ci + 1) * B],
                        start=(ci == 0),
                        stop=(ci == JCH - 1),
                    )
                nc.vector.tensor_copy(zpart, psum_z[:])

            # Z all-reduce
            nc.gpsimd.dma_start(zin[:], zpart)
            nc.gpsimd.collective_compute(
                "AllReduce",
                mybir.AluOpType.add,
                replica_groups=groups,
                ins=[zin.opt()],
                outs=[zout.opt()],
            )
            nc.gpsimd.dma_start(zrow, zout[:])

            # ---- E scale by interp/Z ----
            zinv = rows[:, B : 2 * B]  # reuse zpart slot (already consumed)
            nc.vector.reciprocal(zinv, zrow)
            scale_row = rows[:, 0:B]  # in-place over interp_row (last use)
            nc.vector.tensor_tensor(scale_row, interp_row, zinv, OP.mult)
            scale_bc = cpool.tile([128, B], F32)
            with tc.tile_pool(name="bcps", bufs=1, space="PSUM") as bcps:
                psum_sc = bcps.tile([128, B], F32)
                nc.tensor.matmul(
                    psum_sc[:], lhsT=ones128, rhs=scale_row, start=True, stop=True
                )
                nc.scalar.activation(scale_bc[:], psum_sc[:], AF.Copy)
            for ci in range(JCH):
                sl = slice(ci * B, (ci + 1) * B)
                nc.vector.tensor_tensor(
                    E_all[:, sl], E_all[:, sl], scale_bc[:], OP.mult
                )

            # ---- load exchanged hist: H_all[lo, j*B + p*BSH + b] ----
            hq_cm = tc.tile_pool(name="hq", bufs=1)
            hq = hq_cm.__enter__()
            H_all = hq.tile([128, RCH * B], F32)
            for j in range(RCH):
                nc.sync.dma_start(
                    out=H_all[:, j * B : (j + 1) * B].rearrange(
                        "p (pp b) -> p pp b", b=BSH
                    ),
                    in_=a2a_out[:].rearrange("pp (j lo) b -> lo pp j b", lo=128)[
                        :, :, j, :
                    ],
                )

            # ---- staircase Q matmuls into output tiles ----
            # Q tiles are prebuilt once per chunk covering all v-tiles that
            # touch it; pool rotation retires them once the window passes.
            with (
                tc.tile_pool(
                    name="qg", bufs=max(w for _, w in wins_g) + 2
                ) as qgpool,
                tc.tile_pool(
                    name="qp", bufs=max(w for _, w in wins_p) + 2
                ) as qppool,
                tc.tile_pool(name="opool", bufs=3) as opool,
                tc.tile_pool(name="opsum", bufs=2, space="PSUM") as opsum,
            ):
                qg_tiles = {}
                qp_tiles = {}

                def get_q(ci, tiles, pool, tbl, tmn, tmx, wmax):
                    if ci not in tiles:
                        nt = tmx[ci] - tmn[ci] + 1
                        qt = pool.tile([128, wmax * 128], F32, tag="q")
                        anchor = tmn[ci] * 128
                        nc.vector.tensor_scalar(
                            qt[:, : nt * 128],
                            qiota[:, : nt * 128],
                            tbl[:, ci : ci + 1],
                            float(-anchor),
                            OP.subtract,
                            OP.is_equal,
                        )
                        tiles[ci] = (qt, anchor)
                    return tiles[ci]

                for t in range(NT):
                    psum_o = opsum.tile([128, B], F32)
                    bg0, wgn = wins_g[t]
                    bp0, wpn = wins_p[t]
                    total = wgn + wpn
                    idx = 0
                    for w in range(wgn):
                        ci = bg0 + w
                        qt, anchor = get_q(
                            ci, qg_tiles, qgpool, g2o_sb, g_tmin, g_tmax, gq_w
                        )
                        off = t * 128 - anchor
                        nc.tensor.matmul(
                            psum_o[:],
                            lhsT=qt[:, off : off + 128],
                            rhs=E_all[:, ci * B : (ci + 1) * B],
                            start=(idx == 0),
                            stop=(idx == total - 1),
                        )
                        idx += 1
                    for w in range(wpn):
                        ci = bp0 + w
                        qt, anchor = get_q(
                            ci, qp_tiles, qppool, ip2o_sb, p_tmin, p_tmax, pq_w
                        )
                        off = t * 128 - anchor
                        nc.tensor.matmul(
                            psum_o[:],
                            lhsT=qt[:, off : off + 128],
                            rhs=H_all[:, ci * B : (ci + 1) * B],
                            start=(idx == 0),
                            stop=(idx == total - 1),
                        )
                        idx += 1
                    ot = opool.tile([128, B], F32)
                    if wgn + wpn == 0:
                        nc.vector.memset(ot[:], 0.0)
                    else:
                        nc.scalar.activation(ot[:], psum_o[:], AF.Copy)
                    nc.sync.dma_start(
                        out=out_T[t * 128 : (t + 1) * 128, :], in_=ot[:]
                    )
            hq_cm.__exit__(None, None, None)
    return nc


def split_excess_waits(nc, limit=1):
    """Walrus codegen rejects instructions carrying several sem waits.
    Move excess waits onto preceding no-ops on the same engine."""
    n = 0
    for f in nc.m.functions:
        for bb in f.blocks:
            out = []
            for ins in bb.instructions:
                si = ins.sync_info
                if si is not None and si.on_wait and len(si.on_wait) > limit:
                    waits = list(si.on_wait)
                    for i in range(0, len(waits), limit):
                        nop = mybir.InstNoOp(
                            name=f"I-wsplit-{n}",
                            engine=ins.engine,
                            bass_nofuse=True,
                            sync_info=mybir.SyncInfo(
                                on_wait=waits[i : i + limit], on_update=[]
                            ),
                        )
                        n += 1
                        out.append(nop)
                    si.on_wait.clear()
                out.append(ins)
            bb.instructions[:] = out
    return n


# ============================ host orchestration ============================


def make_in_maps(plan, x, scores, Wp, bp, Wg, bg, ctx_inp, gen_to_out, inp_to_out):
    c = plan["cfg"]
    B, D, S = c["B"], c["D"], c["S"]
    BSH, DC = c["BSH"], c["DC"]
    JCH, RCH = plan["JCH"], plan["RCH"]
    cuts = plan["cuts"]
    pg = plan["gen_perm"]
    g2o_s = plan["g2o_s"]
    ip2o_s = plan["ip2o_s"]
    rank = plan["rank"]

    x = np.asarray(x, dtype=np.float32)
    scores = np.asarray(scores, dtype=np.float32)
    Wg_s = np.asarray(Wg, dtype=np.float32)[:, pg]
    bg_s = np.asarray(bg, dtype=np.float32)[pg]

    x_T = np.ascontiguousarray(x.T)
    wp_ch = np.ascontiguousarray(
        np.asarray(Wp, dtype=np.float32).reshape(DC, 128).T
    )
    bp_t = np.asarray(bp, dtype=np.float32).reshape(1, 1)

    import ml_dtypes

    ctx_rank = rank[np.asarray(ctx_inp)]  # (B, S) int
    chi = (ctx_rank >> 7).astype(ml_dtypes.bfloat16)
    clo = (ctx_rank & 127).astype(ml_dtypes.bfloat16)

    in_maps = []
    for k in range(NCORES):
        m = {"x_T": x_T, "wp_ch": wp_ch, "bp": bp_t}
        bs = slice(k * BSH, (k + 1) * BSH)
        m["x_T64"] = np.ascontiguousarray(x[bs].T)
        m["sco_T"] = np.ascontiguousarray(scores[bs].T)
        m["clo_T"] = np.ascontiguousarray(clo[bs].T)
        m["chi_T"] = np.ascontiguousarray(chi[bs].T)

        # gen shard
        j0, j1 = plan["gen_j0"][k], plan["gen_j1"][k]
        lo = plan["gen_c0"][k] * 128
        wgk = np.zeros((D, JCH * 128), dtype=np.float32)
        bgk = np.full(JCH * 128, -1e9, dtype=np.float32)
        g2k = np.full((128, JCH), 1e9, dtype=np.float32)
        hi = min(lo + JCH * 128, len(g2o_s))
        wgk[:, : hi - lo] = Wg_s[:, lo:hi]
        # bias: valid only inside [j0, j1)
        va, vb = max(j0 - lo, 0), max(j1 - lo, 0)
        bgk[va:vb] = bg_s[lo + va : lo + vb]
        bgk = np.ascontiguousarray(bgk.reshape(JCH, 128).T)
        tmp = np.full(JCH * 128, 1e9, dtype=np.float32)
        tmp[: hi - lo] = (g2o_s[lo:hi] - cuts[k]).astype(np.float32)
        g2k = np.ascontiguousarray(tmp.reshape(JCH, 128).T)
        m["wg"], m["bg_ch"], m["g2o"] = wgk, bgk, g2k

        # ptr shard table
        rlo = plan["ptr_rc0"][k] * 128
        rhi = min(rlo + RCH * 128, len(ip2o_s))
        tmp = np.full(RCH * 128, 1e9, dtype=np.float32)
        tmp[: rhi - rlo] = (ip2o_s[rlo:rhi] - cuts[k]).astype(np.float32)
        m["ip2o"] = np.ascontiguousarray(tmp.reshape(RCH, 128).T)
        in_maps.append(m)
    return in_maps


def assemble_output(plan, results):
    c = plan["cfg"]
    B, OUT_V, VS = c["B"], c["OUT_V"], c["VS"]
    out = np.empty((B, OUT_V), dtype=np.float32)
    for k in range(NCORES):
        v0 = k * VS
        n = min(VS, OUT_V - v0)
        if n <= 0:
            break
        out[:, v0 : v0 + n] = results[k]["out_T"][:n].T
    return out


_CACHE = {}


def kernel(x, scores, Wp, bp, Wg, bg, ctx_inp, gen_to_out, inp_to_out):
    gen_to_out = np.asarray(gen_to_out)
    inp_to_out = np.asarray(inp_to_out)
    plan = make_plan(FULL_CFG, gen_to_out, inp_to_out)
    key = (plan["JCH"], plan["RCH"], tuple(plan["wins_g"]), tuple(plan["wins_p"]))
    if key not in _CACHE:
        nc = build_module(plan)
        split_excess_waits(nc)
        _CACHE[key] = nc
    nc = _CACHE[key]
    in_maps = make_in_maps(
        plan, x, scores, Wp, bp, Wg, bg, ctx_inp, gen_to_out, inp_to_out
    )
    res = run_bass_kernel_spmd(nc, in_maps, list(range(NCORES)))
    return assemble_output(plan, res.results)


if __name__ == "__main__":
    import reference

    inputs = reference.setup_inputs()
    inputs = {k: np.asarray(v) for k, v in inputs.items()}
    out = kernel(**inputs)
    exp = np.asarray(reference.reference(**{k: v for k, v in inputs.items()}))
    err = np.abs(out - exp).max() / (np.abs(exp).max() + 1e-30)
    print("rel err:", err)


# revision 24
# speedup vs baseline: 1.7509x; 1.1554x over previous
"""Pointer-generator output layer on 8 Trainium2 NeuronCores (Bass/Tile).

Algorithm (per reference.py):
  interp = sigmoid(x @ Wp + bp)                              (B, 1)
  gen    = softmax(x @ Wg + bg)  scattered via gen_to_out    (B, OUT_V)
  ptr    = softmax(scores)       scattered via inp_to_out[ctx_inp]
  out    = interp * gen + (1 - interp) * ptr

Sharding: output-vocab sharded across 8 cores. Host sorts the (static,
shared) index tables so both scatters become monotone "staircase" 0/1
matmuls on device:
  - Wg columns permuted by argsort(gen_to_out): each core reads only the
    Wg columns mapping into its vocab slice; softmax stats all-reduced.
  - inp_to_out sorted; ctx_inp relabeled by rank (host int gather). On
    device a per-row histogram over the 20000 rank bins is built with
    one-hot matmuls accumulating in PSUM (exact fp32), batch-sharded;
    the per-rank histograms are exchanged (AllToAll) and applied through
    the sorted inp_to_out staircase matmul into the same PSUM tiles as
    the gen part.

The device does all floating-point work: both matmuls, softmaxes, both
scatters, gating and the final combine. Host work is layout only
(transposes, sorted slicing/padding, integer rank relabel) plus the
final transpose-gather of the output.
"""

import math

import numpy as np

import concourse.bass as bass
import concourse.mybir as mybir
import concourse.tile as tile
from concourse.bass_utils import run_bass_kernel_spmd

F32 = mybir.dt.float32

# -------------------- problem dims (hardcoded per spec) --------------------
FULL_CFG = dict(B=512, D=1024, S=2048, GEN_V=30000, INP_V=20000, OUT_V=50257)
NCORES = 8


def _derive(cfg):
    d = dict(cfg)
    d["VS"] = int(math.ceil(cfg["OUT_V"] / NCORES / 128)) * 128  # v-slice/core
    d["NT"] = d["VS"] // 128  # v-tiles per core
    d["BSH"] = cfg["B"] // NCORES  # batch rows per core (ptr part)
    d["RTOT"] = (cfg["INP_V"] + 127) // 128  # hist hi-chunks (global)
    d["SC"] = cfg["S"] // 128  # score chunks
    d["DC"] = cfg["D"] // 128  # feature chunks
    return d


# ============================ host-side planning ============================


def make_plan(cfg, gen_to_out, inp_to_out):
    """Static plan: sorted tables, per-core shard windows, staircase bases."""
    c = _derive(cfg)
    VS, NT = c["VS"], c["NT"]
    plan = {"cfg": c}

    # ---- gen table sort ----
    pg = np.argsort(gen_to_out, kind="stable")
    g2o_s = gen_to_out[pg].astype(np.int64)
    plan["gen_perm"] = pg
    cuts = np.array([k * VS for k in range(NCORES + 1)], dtype=np.int64)
    j0 = np.searchsorted(g2o_s, cuts[:-1], side="left")
    j1 = np.searchsorted(g2o_s, cuts[1:], side="left")
    c0 = j0 // 128
    spans = (j1 + 127) // 128 - c0
    JCH = max(int(spans.max()), 1)
    plan["JCH"], plan["gen_c0"], plan["gen_j0"], plan["gen_j1"] = JCH, c0, j0, j1

    # local (per-core) sorted tables for staircase bases
    g2o_loc = np.full((NCORES, JCH * 128), np.inf, dtype=np.float64)
    for k in range(NCORES):
        lo = c0[k] * 128
        hi = min(lo + JCH * 128, len(g2o_s))
        g2o_loc[k, : hi - lo] = g2o_s[lo:hi] - cuts[k]
    plan["wins_g"], plan["span_g"] = _staircase_base(g2o_loc, NT, JCH)

    # ---- ptr table sort ----
    pp = np.argsort(inp_to_out, kind="stable")
    ip2o_s = inp_to_out[pp].astype(np.int64)
    rank = np.empty_like(pp)
    rank[pp] = np.arange(len(pp))
    plan["rank"] = rank
    r0 = np.searchsorted(ip2o_s, cuts[:-1], side="left")
    r1 = np.searchsorted(ip2o_s, cuts[1:], side="left")
    rc0 = r0 // 128
    rspans = (r1 + 127) // 128 - rc0
    RCH = max(int(rspans.max()), 1)
    plan["RCH"], plan["ptr_rc0"] = RCH, rc0
    plan["RTOT_PAD"] = max(int(rc0.max()) + RCH, c["RTOT"])
    ip2o_loc = np.full((NCORES, RCH * 128), np.inf, dtype=np.float64)
    for k in range(NCORES):
        lo = rc0[k] * 128
        hi = min(lo + RCH * 128, len(ip2o_s))
        ip2o_loc[k, : hi - lo] = ip2o_s[lo:hi] - cuts[k]
    plan["wins_p"], plan["span_p"] = _staircase_base(ip2o_loc, NT, RCH)

    plan["ip2o_s"] = ip2o_s
    plan["g2o_s"] = g2o_s
    plan["cuts"] = cuts
    return plan


def _staircase_base(tbl_loc, NT, NCH):
    """Shared per-tile base chunk + window width covering all cores.

    tbl_loc: (NCORES, NCH*128) ascending local v values (inf = pad)."""
    base = np.zeros(NT, dtype=np.int64)
    need_hi = np.zeros(NT, dtype=np.int64)
    any_t = np.zeros(NT, dtype=bool)
    clo_all = np.full((NCORES, NT), NCH, dtype=np.int64)
    chi_all = np.full((NCORES, NT), -1, dtype=np.int64)
    for k in range(NCORES):
        a = tbl_loc[k]
        lo = np.searchsorted(a, [t * 128 for t in range(NT)], side="left")
        hi = np.searchsorted(a, [(t + 1) * 128 for t in range(NT)], side="left")
        for t in range(NT):
            if hi[t] > lo[t]:
                clo_all[k, t] = lo[t] // 128
                chi_all[k, t] = (hi[t] - 1) // 128
                any_t[t] = True
    # per-tile union window over cores: [base[t], need_hi[t]] inclusive
    for t in range(NT):
        if any_t[t]:
            base[t] = clo_all[:, t].min()
            need_hi[t] = chi_all[:, t].max()
        else:
            base[t] = 0
            need_hi[t] = -1  # empty window -> no MMs for this tile
    wins = [
        (int(base[t]), int(need_hi[t]) - int(base[t]) + 1) for t in range(NT)
    ]
    # tile span per chunk (for one-shot Q prebuild): chunk ci is used by
    # tiles t in [tmin[ci], tmax[ci]]
    tmin = {}
    tmax = {}
    for t, (b0, w) in enumerate(wins):
        for ci in range(b0, b0 + w):
            tmin[ci] = min(tmin.get(ci, t), t)
            tmax[ci] = max(tmax.get(ci, t), t)
    return wins, (tmin, tmax)


# ============================ device module ============================


def build_module(plan, dtype_build=mybir.dt.bfloat16):
    c = plan["cfg"]
    B, D, S = c["B"], c["D"], c["S"]
    VS, NT, BSH = c["VS"], c["NT"], c["BSH"]
    RTOT, SC, DC = c["RTOT"], c["SC"], c["DC"]
    RTP = plan["RTOT_PAD"]
    JCH, RCH = plan["JCH"], plan["RCH"]
    wins_g, (g_tmin, g_tmax) = plan["wins_g"], plan["span_g"]
    wins_p, (p_tmin, p_tmax) = plan["wins_p"], plan["span_p"]
    rc0 = plan["ptr_rc0"]
    groups = [list(range(NCORES))]
    DTB = dtype_build
    # widest per-chunk Q prebuild (in tiles of 128)
    gq_w = max(g_tmax[ci] - g_tmin[ci] + 1 for ci in g_tmin) if g_tmin else 1
    pq_w = max(p_tmax[ci] - p_tmin[ci] + 1 for ci in p_tmin) if p_tmin else 1
    QW = max(gq_w, pq_w)

    nc = bass.Bass()
    dp = nc.declare_dram_parameter
    x_T = dp("x_T", [D, B], F32, isOutput=False)
    x_T64 = dp("x_T64", [D, BSH], F32, isOutput=False)
    wp_ch = dp("wp_ch", [128, DC], F32, isOutput=False)
    bp_t = dp("bp", [1, 1], F32, isOutput=False)
    wg = dp("wg", [JCH, 128, DC * 128], F32, isOutput=False)
    bg_c = dp("bg_ch", [128, JCH], F32, isOutput=False)
    g2o = dp("g2o", [128, JCH], F32, isOutput=False)
    ip2o = dp("ip2o", [128, RCH], F32, isOutput=False)
    sco_T = dp("sco_T", [S, BSH], F32, isOutput=False)
    clo_T = dp("clo_T", [S, BSH], DTB, isOutput=False)
    chi_T = dp("chi_T", [S, BSH], DTB, isOutput=False)
    out_T = dp("out_T", [VS, B], F32, isOutput=True)

    AF = mybir.ActivationFunctionType
    OP = mybir.AluOpType

    with tile.TileContext(nc) as tc:
        with (
            tc.tile_pool(name="const", bufs=1) as cpool,
            tc.tile_pool(name="dram", bufs=1, space="DRAM") as dram,
        ):
            zin = dram.tile([1, B], F32, tag="zin")
            zout = dram.tile([1, B], F32, tag="zout")
            a2a_in = dram.tile([NCORES, RCH * 128, BSH], F32, tag="a2a_in")
            a2a_out = dram.tile([NCORES, RCH * 128, BSH], F32, tag="a2a_out")

            # ---- constants ----
            consts = cpool.tile([128, 128 + RTOT + 1], F32)
            iota128 = consts[:, 0:128]
            iotaR = consts[:, 128 : 128 + RTOT]
            ones_col = consts[:, 128 + RTOT : 128 + RTOT + 1]
            nc.gpsimd.iota(
                iota128, pattern=[[1, 128]], channel_multiplier=0,
                allow_small_or_imprecise_dtypes=True,
            )
            nc.gpsimd.iota(
                iotaR, pattern=[[1, RTOT]], channel_multiplier=0,
                allow_small_or_imprecise_dtypes=True,
            )
            nc.vector.memset(ones_col, 1.0)
            constb = cpool.tile([128, 128 + RTOT], DTB)
            iota128b = constb[:, 0:128]
            iotaRb = constb[:, 128 : 128 + RTOT]
            nc.vector.tensor_copy(iota128b, iota128)
            nc.vector.tensor_copy(iotaRb, iotaR)
            qiota = cpool.tile([128, QW * 128], F32)
            nc.gpsimd.iota(
                qiota[:], pattern=[[1, QW * 128]], channel_multiplier=0,
                allow_small_or_imprecise_dtypes=True,
            )

            rows = cpool.tile([1, 3 * B + 2 * BSH + 128], F32)
            ones128 = rows[:, 3 * B + 2 * BSH : 3 * B + 2 * BSH + 128]
            nc.vector.memset(ones128, 1.0)
            interp_row = rows[:, 0:B]
            zpart = rows[:, B : 2 * B]
            zrow = rows[:, 2 * B : 3 * B]
            interp64 = rows[:, 3 * B : 3 * B + BSH]
            E_all = cpool.tile([128, JCH * B], F32)
            tbls = cpool.tile([128, 2 * JCH + RCH], F32)
            g2o_sb = tbls[:, 0:JCH]
            nc.sync.dma_start(out=g2o_sb, in_=g2o[:])
            ip2o_sb = tbls[:, JCH : JCH + RCH]
            nc.sync.dma_start(out=ip2o_sb, in_=ip2o[:])
            bg_sb = tbls[:, JCH + RCH : 2 * JCH + RCH]
            nc.sync.dma_start(out=bg_sb, in_=bg_c[:])

            # ---- interp (needed by hist scale + gen scale) ----
            with tc.tile_pool(name="xw", bufs=1) as xw:
                wp_sb = xw.tile([128, DC], F32)
                nc.sync.dma_start(out=wp_sb[:], in_=wp_ch[:])
                bp_sb = xw.tile([1, 1], F32)
                nc.sync.dma_start(out=bp_sb[:], in_=bp_t[:])
                xts = cpool.tile([128, DC * B], F32)
                nc.sync.dma_start(
                    out=xts[:].rearrange("p (dc b) -> p dc b", b=B),
                    in_=x_T[:].rearrange("(dc p) b -> p dc b", p=128),
                )
                xts64 = xw.tile([128, DC * BSH], F32)
                nc.sync.dma_start(
                    out=xts64[:].rearrange("p (dc b) -> p dc b", b=BSH),
                    in_=x_T64[:].rearrange("(dc p) b -> p dc b", p=128),
                )
                with tc.tile_pool(name="ppsum", bufs=1, space="PSUM") as ppsum:
                    psum_i = ppsum.tile([1, B], F32)
                    for dc in range(DC):
                        nc.tensor.matmul(
                            psum_i[:],
                            lhsT=wp_sb[:, dc : dc + 1],
                            rhs=xts[:, dc * B : (dc + 1) * B],
                            start=(dc == 0),
                            stop=(dc == DC - 1),
                        )
                    nc.scalar.activation(
                        interp_row, psum_i[:], AF.Sigmoid, bias=bp_sb[:, :]
                    )
                    psum_i64 = ppsum.tile([1, BSH], F32)
                    for dc in range(DC):
                        nc.tensor.matmul(
                            psum_i64[:],
                            lhsT=wp_sb[:, dc : dc + 1],
                            rhs=xts64[:, dc * BSH : (dc + 1) * BSH],
                            start=(dc == 0),
                            stop=(dc == DC - 1),
                        )
                    nc.scalar.activation(
                        interp64, psum_i64[:], AF.Sigmoid, bias=bp_sb[:, :]
                    )

            # ---- hist (DVE-heavy) interleaved with logits (PE/DMA-heavy) ----
            with (
                tc.tile_pool(name="hist", bufs=1) as hist,
                tc.tile_pool(name="wbuild", bufs=2) as wbuild,
                tc.tile_pool(name="ubuild", bufs=2) as ubuild,
                tc.tile_pool(name="hpsum", bufs=2, space="PSUM") as hpsum,
                tc.tile_pool(name="zspsum", bufs=1, space="PSUM") as zspsum,
                tc.tile_pool(name="wgp", bufs=3) as wgp,
                tc.tile_pool(name="lpsum", bufs=2, space="PSUM") as lpsum,
                tc.tile_pool(name="zpsum", bufs=1, space="PSUM") as zpsum,
            ):
                sco_sb = hist.tile([128, SC * BSH], F32)
                nc.sync.dma_start(
                    out=sco_sb[:].rearrange("p (sc b) -> p sc b", b=BSH),
                    in_=sco_T[:].rearrange("(sc p) b -> p sc b", p=128),
                )
                clo_sb = hist.tile([128, SC * BSH], DTB)
                nc.sync.dma_start(
                    out=clo_sb[:].rearrange("p (sc b) -> p sc b", b=BSH),
                    in_=clo_T[:].rearrange("(sc p) b -> p sc b", p=128),
                )
                chi_sb = hist.tile([128, SC * BSH], DTB)
                nc.sync.dma_start(
                    out=chi_sb[:].rearrange("p (sc b) -> p sc b", b=BSH),
                    in_=chi_T[:].rearrange("(sc p) b -> p sc b", p=128),
                )
                A_all = hist.tile([128, SC * BSH], F32)
                nc.scalar.activation(A_all[:], sco_sb[:], AF.Exp)
                hrows = hist.tile([1, 4 * BSH], F32)
                zs_inv = hrows[:, 0:BSH]
                cs_row = hrows[:, BSH : 2 * BSH]
                psum_zs = zspsum.tile([1, BSH], F32)
                for sc in range(SC):
                    nc.tensor.matmul(
                        psum_zs[:],
                        lhsT=ones_col,
                        rhs=A_all[:, sc * BSH : (sc + 1) * BSH],
                        start=(sc == 0),
                        stop=(sc == SC - 1),
                    )
                zs_sb = hrows[:, 2 * BSH : 3 * BSH]
                nc.vector.tensor_copy(zs_sb, psum_zs[:])
                nc.vector.reciprocal(zs_inv, zs_sb)
                om_row = hrows[:, 3 * BSH : 4 * BSH]
                nc.vector.tensor_scalar(
                    om_row, interp64, -1.0, 1.0, OP.mult, OP.add
                )
                nc.vector.tensor_tensor(cs_row, om_row, zs_inv, OP.mult)
                cs_bc = hist.tile([128, BSH], F32)
                psum_bc = zspsum.tile([128, BSH], F32, tag="bc")
                nc.tensor.matmul(
                    psum_bc[:], lhsT=ones128, rhs=cs_row, start=True, stop=True
                )
                nc.scalar.activation(cs_bc[:], psum_bc[:], AF.Copy)
                Ab = hist.tile([128, SC * BSH], DTB)
                for sc in range(SC):
                    sl = slice(sc * BSH, (sc + 1) * BSH)
                    nc.vector.tensor_tensor(
                        Ab[:, sl], A_all[:, sl], cs_bc[:], OP.mult
                    )

                HT = hist.tile([128, RTP * BSH], F32)
                if RTP > RTOT:
                    nc.vector.memset(HT[:, RTOT * BSH : RTP * BSH], 0.0)
                psum_z = zpsum.tile([1, B], F32)

                def emit_logits_chunk(ci):
                    wgt = wgp.tile([128, DC * 128], F32)
                    nc.sync.dma_start(out=wgt[:], in_=wg[ci])
                    psum_l = lpsum.tile([128, B], F32)
                    for dc in range(DC):
                        nc.tensor.matmul(
                            psum_l[:],
                            lhsT=wgt[:, dc * 128 : (dc + 1) * 128],
                            rhs=xts[:, dc * B : (dc + 1) * B],
                            start=(dc == 0),
                            stop=(dc == DC - 1),
                        )
                    nc.scalar.activation(
                        E_all[:, ci * B : (ci + 1) * B],
                        psum_l[:],
                        AF.Exp,
                        bias=bg_sb[:, ci : ci + 1],
                    )
                    nc.tensor.matmul(
                        psum_z[:],
                        lhsT=ones_col,
                        rhs=E_all[:, ci * B : (ci + 1) * B],
                        start=(ci == 0),
                        stop=(ci == JCH - 1),
                    )

                def emit_hist_row(b):
                    Wt = wbuild.tile([128, SC * 128], DTB)
                    lo_b = (
                        clo_sb[:]
                        .rearrange("p (sc b) -> p sc b", b=BSH)[:, :, b : b + 1]
                        .to_broadcast([128, SC, 128])
                    )
                    nc.vector.tensor_tensor(
                        Wt[:].rearrange("p (sc j) -> p sc j", j=128),
                        lo_b,
                        iota128b.rearrange("p (o j) -> p o j", o=1).to_broadcast(
                            [128, SC, 128]
                        ),
                        OP.is_equal,
                    )
                    a_b = (
                        Ab[:]
                        .rearrange("p (sc b) -> p sc b", b=BSH)[:, :, b : b + 1]
                        .to_broadcast([128, SC, 128])
                    )
                    nc.vector.tensor_tensor(
                        Wt[:].rearrange("p (sc j) -> p sc j", j=128),
                        Wt[:].rearrange("p (sc j) -> p sc j", j=128),
                        a_b,
                        OP.mult,
                    )
                    Ut = ubuild.tile([128, SC * RTOT], DTB)
                    hi_b = (
                        chi_sb[:]
                        .rearrange("p (sc b) -> p sc b", b=BSH)[:, :, b : b + 1]
                        .to_broadcast([128, SC, RTOT])
                    )
                    nc.vector.tensor_tensor(
                        Ut[:].rearrange("p (sc j) -> p sc j", j=RTOT),
                        hi_b,
                        iotaRb.rearrange("p (o j) -> p o j", o=1).to_broadcast(
                            [128, SC, RTOT]
                        ),
                        OP.is_equal,
                    )
                    psum_h = hpsum.tile([128, RTOT], F32)
                    for sc in range(SC):
                        nc.tensor.matmul(
                            psum_h[:],
                            lhsT=Wt[:, sc * 128 : (sc + 1) * 128],
                            rhs=Ut[:, sc * RTOT : (sc + 1) * RTOT],
                            start=(sc == 0),
                            stop=(sc == SC - 1),
                        )
                    nc.vector.tensor_copy(
                        HT[:].rearrange("p (c b) -> p c b", b=BSH)[:, 0:RTOT, b],
                        psum_h[:],
                    )

                for i in range(max(BSH, JCH)):
                    if i < JCH:
                        emit_logits_chunk(i)
                    if i < BSH:
                        emit_hist_row(i)
                nc.vector.tensor_copy(zpart, psum_z[:])

                # ---- hist exchange (AllToAll); slabs in [lo, j, b] layout ----
                for p in range(NCORES):
                    nc.sync.dma_start(
                        out=a2a_in[:][p].rearrange("(lo j) b -> lo j b", lo=128),
                        in_=HT[:, rc0[p] * BSH : (rc0[p] + RCH) * BSH],
                    )
                nc.gpsimd.collective_compute(
                    "AllToAll",
                    mybir.AluOpType.bypass,
                    replica_groups=groups,
                    ins=[a2a_in.opt()],
                    outs=[a2a_out.opt()],
                )

            # Z all-reduce
            nc.gpsimd.dma_start(zin[:], zpart)
            nc.gpsimd.collective_compute(
                "AllReduce",
                mybir.AluOpType.add,
                replica_groups=groups,
                ins=[zin.opt()],
                outs=[zout.opt()],
            )
            nc.gpsimd.dma_start(zrow, zout[:])

            # ---- E scale by interp/Z ----
            zinv = rows[:, B : 2 * B]  # reuse zpart slot (already consumed)
            nc.vector.reciprocal(zinv, zrow)
            scale_row = rows[:, 0:B]  # in-place over interp_row (last use)
            nc.vector.tensor_tensor(scale_row, interp_row, zinv, OP.mult)
            scale_bc = cpool.tile([128, B], F32)
            with tc.tile_pool(name="bcps", bufs=1, space="PSUM") as bcps:
                psum_sc = bcps.tile([128, B], F32)
                nc.tensor.matmul(
                    psum_sc[:], lhsT=ones128, rhs=scale_row, start=True, stop=True
                )
                nc.scalar.activation(scale_bc[:], psum_sc[:], AF.Copy)
            for ci in range(JCH):
                sl = slice(ci * B, (ci + 1) * B)
                nc.vector.tensor_tensor(
                    E_all[:, sl], E_all[:, sl], scale_bc[:], OP.mult
                )

            # ---- load exchanged hist: H_all[lo, (pp, j, b)] (8 contiguous DMAs)
            hq_cm = tc.tile_pool(name="hq", bufs=1)
            hq = hq_cm.__enter__()
            H_all = hq.tile([128, NCORES * RCH * BSH], F32)
            for pp in range(NCORES):
                nc.sync.dma_start(
                    out=H_all[
                        :, pp * RCH * BSH : (pp + 1) * RCH * BSH
                    ].rearrange("p (j b) -> p j b", b=BSH),
                    in_=a2a_out[:][pp].rearrange("(lo j) b -> lo j b", lo=128),
                )

            # ---- staircase Q matmuls into output tiles ----
            # Q tiles are prebuilt once per chunk covering all v-tiles that
            # touch it; pool rotation retires them once the window passes.
            with (
                tc.tile_pool(
                    name="qg", bufs=max(w for _, w in wins_g) + 6
                ) as qgpool,
                tc.tile_pool(
                    name="qp", bufs=max(w for _, w in wins_p) + 6
                ) as qppool,
                tc.tile_pool(name="opool", bufs=3) as opool,
                tc.tile_pool(name="opsum", bufs=2, space="PSUM") as opsum,
            ):
                qg_tiles = {}
                qp_tiles = {}

                def get_q(ci, tiles, pool, tbl, tmn, tmx, wmax):
                    if ci not in tiles:
                        nt = tmx[ci] - tmn[ci] + 1
                        qt = pool.tile([128, wmax * 128], F32, tag="q")
                        anchor = tmn[ci] * 128
                        nc.vector.tensor_scalar(
                            qt[:, : nt * 128],
                            qiota[:, : nt * 128],
                            tbl[:, ci : ci + 1],
                            float(-anchor),
                            OP.subtract,
                            OP.is_equal,
                        )
                        tiles[ci] = (qt, anchor)
                    return tiles[ci]

                for t in range(NT):
                    psum_o = opsum.tile([128, B], F32)
                    bg0, wgn = wins_g[t]
                    bp0, wpn = wins_p[t]
                    total = wgn + wpn
                    idx = 0
                    for w in range(wgn):
                        ci = bg0 + w
                        qt, anchor = get_q(
                            ci, qg_tiles, qgpool, g2o_sb, g_tmin, g_tmax, gq_w
                        )
                        off = t * 128 - anchor
                        nc.tensor.matmul(
                            psum_o[:],
                            lhsT=qt[:, off : off + 128],
                            rhs=E_all[:, ci * B : (ci + 1) * B],
                            start=(idx == 0),
                            stop=(idx == total - 1),
                        )
                        idx += 1
                    for w in range(wpn):
                        ci = bp0 + w
                        qt, anchor = get_q(
                            ci, qp_tiles, qppool, ip2o_sb, p_tmin, p_tmax, pq_w
                        )
                        off = t * 128 - anchor
                        nc.tensor.matmul(
                            psum_o[:],
                            lhsT=qt[:, off : off + 128],
                            rhs=H_all[:].rearrange(
                                "p (pp j b) -> p pp j b", pp=NCORES, b=BSH
                            )[:, :, ci, :],
                            start=(idx == 0),
                            stop=(idx == total - 1),
                        )
                        idx += 1
                    ot = opool.tile([128, B], F32)
                    if wgn + wpn == 0:
                        nc.vector.memset(ot[:], 0.0)
                    else:
                        nc.scalar.activation(ot[:], psum_o[:], AF.Copy)
                    nc.sync.dma_start(
                        out=out_T[t * 128 : (t + 1) * 128, :], in_=ot[:]
                    )
            hq_cm.__exit__(None, None, None)
    return nc


def split_excess_waits(nc, limit=1):
    """Walrus codegen rejects instructions carrying several sem waits.
    Move excess waits onto preceding no-ops on the same engine."""
    n = 0
    for f in nc.m.functions:
        for bb in f.blocks:
            out = []
            for ins in bb.instructions:
                si = ins.sync_info
                if si is not None and si.on_wait and len(si.on_wait) > limit:
                    waits = list(si.on_wait)
                    for i in range(0, len(waits), limit):
                        nop = mybir.InstNoOp(
                            name=f"I-wsplit-{n}",
                            engine=ins.engine,
                            bass_nofuse=True,
                            sync_info=mybir.SyncInfo(
                                on_wait=waits[i : i + limit], on_update=[]
                            ),
                        )
                        n += 1
                        out.append(nop)
                    si.on_wait.clear()
                out.append(ins)
            bb.instructions[:] = out
    return n


# ============================ host orchestration ============================


def make_in_maps(plan, x, scores, Wp, bp, Wg, bg, ctx_inp, gen_to_out, inp_to_out):
    c = plan["cfg"]
    B, D, S = c["B"], c["D"], c["S"]
    BSH, DC = c["BSH"], c["DC"]
    JCH, RCH = plan["JCH"], plan["RCH"]
    cuts = plan["cuts"]
    pg = plan["gen_perm"]
    g2o_s = plan["g2o_s"]
    ip2o_s = plan["ip2o_s"]
    rank = plan["rank"]

    x = np.asarray(x, dtype=np.float32)
    scores = np.asarray(scores, dtype=np.float32)
    Wg_s = np.asarray(Wg, dtype=np.float32)[:, pg]
    bg_s = np.asarray(bg, dtype=np.float32)[pg]

    x_T = np.ascontiguousarray(x.T)
    wp_ch = np.ascontiguousarray(
        np.asarray(Wp, dtype=np.float32).reshape(DC, 128).T
    )
    bp_t = np.asarray(bp, dtype=np.float32).reshape(1, 1)

    import ml_dtypes

    ctx_rank = rank[np.asarray(ctx_inp)]  # (B, S) int
    chi = (ctx_rank >> 7).astype(ml_dtypes.bfloat16)
    clo = (ctx_rank & 127).astype(ml_dtypes.bfloat16)

    in_maps = []
    for k in range(NCORES):
        m = {"x_T": x_T, "wp_ch": wp_ch, "bp": bp_t}
        bs = slice(k * BSH, (k + 1) * BSH)
        m["x_T64"] = np.ascontiguousarray(x[bs].T)
        m["sco_T"] = np.ascontiguousarray(scores[bs].T)
        m["clo_T"] = np.ascontiguousarray(clo[bs].T)
        m["chi_T"] = np.ascontiguousarray(chi[bs].T)

        # gen shard
        j0, j1 = plan["gen_j0"][k], plan["gen_j1"][k]
        lo = plan["gen_c0"][k] * 128
        wgk = np.zeros((D, JCH * 128), dtype=np.float32)
        bgk = np.full(JCH * 128, -1e9, dtype=np.float32)
        g2k = np.full((128, JCH), 1e9, dtype=np.float32)
        hi = min(lo + JCH * 128, len(g2o_s))
        wgk[:, : hi - lo] = Wg_s[:, lo:hi]
        # bias: valid only inside [j0, j1)
        va, vb = max(j0 - lo, 0), max(j1 - lo, 0)
        bgk[va:vb] = bg_s[lo + va : lo + vb]
        bgk = np.ascontiguousarray(bgk.reshape(JCH, 128).T)
        tmp = np.full(JCH * 128, 1e9, dtype=np.float32)
        tmp[: hi - lo] = (g2o_s[lo:hi] - cuts[k]).astype(np.float32)
        g2k = np.ascontiguousarray(tmp.reshape(JCH, 128).T)
        # relayout to [JCH, p(=d%128), dc*128+j] for contiguous chunk DMAs
        wgk = np.ascontiguousarray(
            wgk.reshape(DC, 128, JCH, 128).transpose(2, 1, 0, 3).reshape(
                JCH, 128, DC * 128
            )
        )
        m["wg"], m["bg_ch"], m["g2o"] = wgk, bgk, g2k

        # ptr shard table
        rlo = plan["ptr_rc0"][k] * 128
        rhi = min(rlo + RCH * 128, len(ip2o_s))
        tmp = np.full(RCH * 128, 1e9, dtype=np.float32)
        tmp[: rhi - rlo] = (ip2o_s[rlo:rhi] - cuts[k]).astype(np.float32)
        m["ip2o"] = np.ascontiguousarray(tmp.reshape(RCH, 128).T)
        in_maps.append(m)
    return in_maps


def assemble_output(plan, results):
    c = plan["cfg"]
    B, OUT_V, VS = c["B"], c["OUT_V"], c["VS"]
    out = np.empty((B, OUT_V), dtype=np.float32)
    for k in range(NCORES):
        v0 = k * VS
        n = min(VS, OUT_V - v0)
        if n <= 0:
            break
        out[:, v0 : v0 + n] = results[k]["out_T"][:n].T
    return out


_CACHE = {}


def kernel(x, scores, Wp, bp, Wg, bg, ctx_inp, gen_to_out, inp_to_out):
    gen_to_out = np.asarray(gen_to_out)
    inp_to_out = np.asarray(inp_to_out)
    plan = make_plan(FULL_CFG, gen_to_out, inp_to_out)
    key = (plan["JCH"], plan["RCH"], tuple(plan["wins_g"]), tuple(plan["wins_p"]))
    if key not in _CACHE:
        nc = build_module(plan)
        split_excess_waits(nc)
        _CACHE[key] = nc
    nc = _CACHE[key]
    in_maps = make_in_maps(
        plan, x, scores, Wp, bp, Wg, bg, ctx_inp, gen_to_out, inp_to_out
    )
    res = run_bass_kernel_spmd(nc, in_maps, list(range(NCORES)))
    return assemble_output(plan, res.results)


if __name__ == "__main__":
    import reference

    inputs = reference.setup_inputs()
    inputs = {k: np.asarray(v) for k, v in inputs.items()}
    out = kernel(**inputs)
    exp = np.asarray(reference.reference(**{k: v for k, v in inputs.items()}))
    err = np.abs(out - exp).max() / (np.abs(exp).max() + 1e-30)
    print("rel err:", err)
